# revision 25
# baseline (speedup 1.0000x reference)
"""DeepseekV2 MLA attention prefill on 8 Trainium2 NeuronCores.

Tensor-parallel over heads (ColumnParallel q_b/kv_b + RowParallel o_proj):
 - a-projection column-sharded; hT staged p-major-contiguous and SBUF-
   resident (4KB/partition DMA descriptors); AllGather split into two
   T-halves so norms + the w_q_b half-0 sweep overlap AG half 1. The
   collective path has a ~100us init floor (warmup barrier + ~10us/op),
   so stage A is entirely hidden under it.
 - RMSNorm feature-major (PE ones-matmul reduction) per T-half; ln
   weights folded into the b-projections host-side; norm scale broadcast
   across partitions with a K=1 matmul and applied on psum drains
 - b-projections emit strict psum bank ping-pong (same-bank back-to-back
   matmuls stall ~100ns): the w_q_b sweeps and the v projection pair two
   output tiles in alternating banks; w_kv weights preloaded during the
   AG window so the kn/v projections never starve; q-rope (DVE) emitted
   after the v matmuls so it overlaps them
 - attention: S^T tiles k-major; 2-bank [P,T] score psums (bufs=3), one
   exp ACT per (head, j); nope/rope matmuls batched by type with bank-
   ordered emission; K=64 rope matmuls in disjoint PE row-groups
 - o_proj bf16 (fp8 variants all bust the 2e-2 gate: v fp8 0.028,
   aot+wo fp8 0.040, pts fp8 NaN); wo prefetch depth 6
 - ReduceScatter in bf16, uneven chunks (13/10/8/5/3/1 m-tiles) so comm
   hides under o_proj and the exposed tail is minimal.
"""
import sys

sys.path.insert(0, "/opt/trn_rl_repo")

import numpy as np
import ml_dtypes

import concourse.bass as bass
import concourse.tile as tile
import concourse.mybir as mybir
from concourse import bacc
from concourse.bass_utils import run_bass_kernel_spmd

# model dims
T = 1024
HID = 5120
NH = 128
DN = 128
DR = 64
DV = 128
QLR = 1536
KVLR = 512
EPS = 1e-6
THETA = 10000.0

NC = 8              # cores
HPC = NH // NC      # heads per core = 16
ACOL = (QLR + KVLR + DR) // NC   # a-proj columns per core = 264
SM_SCALE = float((DN + DR) ** -0.5)
CONST_M = 35.0      # constant max-shift for exp (validated in test harness)

P = 128
NCH = 2                 # T-halves for AG/norm/b-proj pipeline
CW = T // NCH           # chunk width = 512
KT = HPC * DV // P      # o_proj k-tiles = 16
NKH = HID // P          # hT k-tiles = 40
RS_MT = [13, 10, 8, 5, 3, 1]       # o_proj m-tiles per RS chunk (sum 40)

F32 = mybir.dt.float32
BF16 = mybir.dt.bfloat16
AF = mybir.ActivationFunctionType
GROUPS = [list(range(NC))]

_CACHE = {}


def build():
    nc = bacc.Bacc("TRN2", target_bir_lowering=False, debug=False, num_devices=NC)

    hT = nc.dram_tensor("hT", [P, NKH, T], BF16, kind="ExternalInput")
    wa = nc.dram_tensor("wa", [P, NKH, ACOL], BF16, kind="ExternalInput")
    cos2d = nc.dram_tensor("cos2d", [P, T], BF16, kind="ExternalInput")
    sin2d = nc.dram_tensor("sin2d", [P, T], BF16, kind="ExternalInput")
    # b-weights p-major: [P, m, k, P] so one big DMA covers many m-tiles
    wqn = nc.dram_tensor("wqn", [P, HPC * DN // P, QLR // P, P], BF16,
                         kind="ExternalInput")
    wqr = nc.dram_tensor("wqr", [P, HPC * DR // P, QLR // P, P], BF16,
                         kind="ExternalInput")
    wkk = nc.dram_tensor("wkk", [P, HPC * DN // P, KVLR // P, P], BF16,
                         kind="ExternalInput")
    wkv = nc.dram_tensor("wkv", [P, KVLR // P, HPC * DV], BF16, kind="ExternalInput")
    wo = nc.dram_tensor("wo", [HID // P, P, KT, P], BF16, kind="ExternalInput")
    triu = nc.dram_tensor("triu", [P, P], BF16, kind="ExternalInput")
    ones = nc.dram_tensor("ones", [P, 1], BF16, kind="ExternalInput")
    out_part = nc.dram_tensor("out_part", [HID // NC, T], BF16, kind="ExternalOutput")

    qkv_c = [nc.dram_tensor(f"qkv_c{n}", [QLR + KVLR + DR, CW], BF16,
                            addr_space="Shared") for n in range(NCH)]
    warm_out = nc.dram_tensor("warm_out", [NC, P], BF16, addr_space="Shared")
    warm_out1 = nc.dram_tensor("warm_out1", [1, P], BF16, addr_space="Shared")

    with tile.TileContext(nc) as tc:
        with (
            tc.tile_pool(name="persist", bufs=1) as pp,
            tc.tile_pool(name="dram", bufs=1, space="DRAM") as dd,
        ):
            ones_t = pp.tile([P, 1], BF16, tag="ones", name="ones")
            nc.sync.dma_start(ones_t[:], ones[:])
            onesr_t = pp.tile([1, P], F32, tag="onesr", name="onesr")
            nc.vector.memset(onesr_t[:], 1.0)
            triu_t = pp.tile([P, P], BF16, tag="triu", name="triu")
            nc.sync.dma_start(triu_t[:], triu[:])
            cos2_t = pp.tile([P, T], BF16, tag="cos2", name="cos2")
            nc.sync.dma_start(cos2_t[:], cos2d[:])
            sin2_t = pp.tile([P, T], BF16, tag="sin2", name="sin2")
            nc.sync.dma_start(sin2_t[:], sin2d[:])
            eps_t = pp.tile([1, 1], F32, tag="epsc", name="epsc")
            nc.vector.memset(eps_t[:], EPS)
            negm_t = pp.tile([P, 1], F32, tag="negm", name="negm")
            nc.vector.memset(negm_t[:], -CONST_M)
            kpe2_t = pp.tile([P, T], BF16, tag="kpe", name="kpe")

            bounce = [dd.tile([ACOL, CW], BF16, tag=f"bn{n}", name=f"bn{n}")
                      for n in range(NCH)]
            warm_in = dd.tile([1, P], BF16, tag="wrm", name="wrm")
            o_dram = [dd.tile([mt * P, T], BF16, tag=f"od{r}", name=f"od{r}")
                      for r, mt in enumerate(RS_MT)]
            rs_out = [dd.tile([mt * P // NC, T], BF16, tag=f"rs{r}", name=f"rs{r}")
                      for r, mt in enumerate(RS_MT)]

            # ---------------- stage A: qkv slice = wa.T @ h ----------------
            with (
                tc.tile_pool(name="stA", bufs=1, side="right") as sa,
                tc.tile_pool(name="psA", bufs=1, space="PSUM") as psa,
            ):
                # tiny collective first: absorbs the ~60us first-collective
                # ncfw warmup barrier so the real AllGathers start promptly
                nc.gpsimd.collective_compute(
                    "AllGather", mybir.AluOpType.bypass,
                    replica_groups=[[c] for c in range(NC)],
                    ins=[warm_in[:]], outs=[warm_out1[:]])
                wa_t = sa.tile([P, NKH, ACOL], BF16, tag="wa", name="wa")
                nc.sync.dma_start(wa_t[:, :NKH // 2, :], wa[:, :NKH // 2, :])
                nc.sync.dma_start(wa_t[:, NKH // 2:, :], wa[:, NKH // 2:, :])
                ht_t = sa.tile([P, NKH, T], BF16, tag="ht", name="ht")
                for kp in range(NKH // 2):
                    nc.sync.dma_start(ht_t[:, 2 * kp:2 * kp + 2, :],
                                      hT[:, 2 * kp:2 * kp + 2, :])
                psums = [psa.tile([P, CW], F32, tag=f"pa{m}{n}", name=f"pa{m}{n}")
                         for m in range(3) for n in range(2)]
                # n-outer: the first T-half finishes its contraction ~30us
                # early, so AG0 launches right at the collective-warmup floor
                for n in range(NCH):
                    for k in range(NKH):
                        for m in range(3):
                            mw = P if m < 2 else ACOL - 2 * P
                            nc.tensor.matmul(
                                psums[2 * m + n][:mw],
                                wa_t[:, k, P * m:P * m + mw],
                                ht_t[:, k, CW * n:CW * (n + 1)],
                                start=(k == 0), stop=(k == NKH - 1))
                    for m in range(3):
                        mw = P if m < 2 else ACOL - 2 * P
                        ot = sa.tile([P, CW], BF16, tag="aout", name="aout",
                                     bufs=3)
                        nc.scalar.copy(ot[:mw], psums[2 * m + n][:mw])
                        nc.sync.dma_start(bounce[n][P * m:P * m + mw, :],
                                          ot[:mw])
                    nc.gpsimd.collective_compute(
                        "AllGather", mybir.AluOpType.bypass,
                        replica_groups=GROUPS,
                        ins=[bounce[n][:]], outs=[qkv_c[n][:]])

            # left stack: long-lived tiles (aot through D; qn/qr/kn/v
            # through C). right stack: transients with shorter lifetimes.
            with tc.tile_pool(name="aotp", bufs=1) as aop:
                aot = aop.tile([P, KT, T], BF16, tag="aot", name="aot")
                with tc.tile_pool(name="qnqr", bufs=1) as mp:
                    qn_t = [mp.tile([P, T], BF16, tag=f"qn{m}", name=f"qn{m}")
                            for m in range(HPC)]
                    qr_t = [mp.tile([P, T], BF16, tag=f"qr{m}", name=f"qr{m}")
                            for m in range(HPC * DR // P)]

                    # kvan + norm scales + preloaded kv b-weights (the DMAs
                    # fly during the AG window, long before their use)
                    apk = tc.alloc_tile_pool(name="apk", bufs=1, side="right")
                    kvan = [apk.tile([P, T], BF16, tag=f"kvan{k}",
                                     name=f"kvan{k}") for k in range(KVLR // P)]
                    rbs = {"q": apk.tile([P, T], F32, tag="rbq", name="rbq"),
                           "kv": apk.tile([P, T], F32, tag="rbkv", name="rbkv")}
                    wkk_t = apk.tile([P, HPC, KVLR // P, P], BF16, tag="wkk",
                                     name="wkk")
                    sn = tc.alloc_tile_pool(name="stN", bufs=2, side="right")

                    def rope_tile(dst, cols, src=None):
                        # rows in 32-groups [e,o,(e,o)]; out = x*cos2 +
                        # swap(x)*sin2 with the signs folded into sin2.
                        ext = src is not None
                        npart = dst.shape[0]
                        w = cols.stop - cols.start
                        rd = (lambda g: src[32 * g:32 * (g + 1), :]) if ext \
                            else (lambda g: dst[32 * g:32 * (g + 1), cols])
                        sw = sn.tile([P, CW], BF16, tag="rsw", name="rsw",
                                     bufs=1)
                        for g in range(npart // 32):
                            nc.sync.dma_start(sw[32 * g:32 * (g + 1), :w],
                                              rd(g ^ 1))
                        t1 = sn.tile([P, CW], BF16, tag="rt1", name="rt1",
                                     bufs=1)
                        nc.vector.tensor_mul(
                            t1[:npart, :w], src[:] if ext else dst[:, cols],
                            cos2_t[:npart, cols])
                        t2 = sn.tile([P, CW], BF16, tag="rt2", name="rt2",
                                     bufs=1)
                        nc.vector.tensor_mul(t2[:npart, :w], sw[:npart, :w],
                                             sin2_t[:npart, cols])
                        nc.vector.tensor_add(dst[:, cols], t1[:npart, :w],
                                             t2[:npart, :w])

                    with tc.tile_pool(name="apq", bufs=1, side="right") as apq:
                        qan = [apq.tile([P, T], BF16, tag=f"qan{k}",
                                        name=f"qan{k}")
                               for k in range(QLR // P)]

                        bw1 = tc.alloc_tile_pool(name="bw1", bufs=1,
                                                 side="right")
                        wqr_t = bw1.tile([P, HPC * DR // P, QLR // P, P],
                                         BF16, tag="wqr", name="wqr")

                        with (
                            tc.tile_pool(name="wqnp", bufs=1,
                                         side="right") as wp,
                            tc.tile_pool(name="psN", bufs=1,
                                         space="PSUM") as psn,
                            tc.tile_pool(name="psQ", bufs=1,
                                         space="PSUM") as psq,
                        ):
                            def norm_chunk(name, nk, row0, dest, dim, cc):
                                # per-T-half rmsnorm scale: rstd via two
                                # alternating psum accumulators + Sqrt + fast
                                # reciprocal; partition broadcast via a K=1
                                # matmul
                                ccs = slice(CW * cc, CW * (cc + 1))
                                s2 = [psn.tile([1, CW], F32, tag=f"s2{p}",
                                               name=f"s2{p}", bufs=1)
                                      for p in range(2)]
                                for k in range(nk):
                                    nc.sync.dma_start(
                                        dest[k][:, ccs],
                                        qkv_c[cc][row0 + P * k:
                                                  row0 + P * (k + 1), :])
                                    sq = sn.tile([P, CW], BF16, tag="sq",
                                                 name="sq", bufs=1)
                                    nc.vector.tensor_mul(sq[:], dest[k][:, ccs],
                                                         dest[k][:, ccs])
                                    nc.tensor.matmul(
                                        s2[k % 2][:], ones_t[:], sq[:],
                                        start=(k < 2), stop=(k >= nk - 2))
                                s2sum = sn.tile([1, CW], F32, tag="s2s",
                                                name="s2s", bufs=1)
                                nc.vector.tensor_copy(s2sum[:], s2[0][:])
                                nc.vector.tensor_add(s2sum[:], s2sum[:],
                                                     s2[1][:])
                                nc.scalar.activation(s2sum[:], s2sum[:],
                                                     AF.Sqrt, bias=eps_t[:],
                                                     scale=1.0 / dim)
                                nc.vector.reciprocal_approx_fast(s2sum[:],
                                                                 s2sum[:])
                                rbp = psn.tile([P, CW], F32, tag="rbp",
                                               name="rbp", bufs=2)
                                nc.tensor.matmul(rbp[:], onesr_t[:], s2sum[:],
                                                 start=True, stop=True)
                                nc.vector.tensor_copy(rbs[name][:, ccs], rbp[:])

                            def kpe_chunk(cc):
                                # k_pe rope (rows 0:64) in place -> replicate
                                ccs = slice(CW * cc, CW * (cc + 1))
                                nc.sync.dma_start(kpe2_t[:DR, ccs],
                                                  qkv_c[cc][QLR + KVLR:, :])
                                rope_tile(kpe2_t[:DR], ccs)
                                nc.sync.dma_start(kpe2_t[DR:2 * DR, ccs],
                                                  kpe2_t[:DR, ccs])

                            def qn_sweep(cc):
                                # w_q_b nope proj for one T-half: grouped
                                # weight loads (4 m-tiles, double-buffered);
                                # two m-tiles accumulate in parallel banks so
                                # consecutive matmuls ping-pong banks
                                ccs = slice(CW * cc, CW * (cc + 1))
                                for g in range(HPC // 4):
                                    wg = wp.tile([P, 4, QLR // P, P], BF16,
                                                 tag="wqn", name="wqn", bufs=2)
                                    nc.sync.dma_start(wg[:],
                                                      wqn[:, 4 * g:4 * (g + 1)])
                                    for mp_ in range(2):
                                        ps = [psq.tile([P, CW], F32,
                                                       tag=f"sw{p}",
                                                       name=f"sw{p}", bufs=2)
                                              for p in range(2)]
                                        for k in range(QLR // P):
                                            for p in range(2):
                                                nc.tensor.matmul(
                                                    ps[p][:],
                                                    wg[:, 2 * mp_ + p, k, :],
                                                    qan[k][:, ccs],
                                                    start=(k == 0),
                                                    stop=(k == QLR // P - 1))
                                        for p in range(2):
                                            m = 4 * g + 2 * mp_ + p
                                            nc.vector.tensor_mul(
                                                qn_t[m][:, ccs], ps[p][:],
                                                rbs["q"][:, ccs])

                            norm_chunk("q", QLR // P, 0, qan, QLR, 0)
                            norm_chunk("kv", KVLR // P, QLR, kvan, KVLR, 0)
                            kpe_chunk(0)
                            qn_sweep(0)
                            # wqr/wkk loads fire here: after AG1's transfer
                            # window, well before their consumers
                            for g in range(2):
                                nc.sync.dma_start(wqr_t[:, 4 * g:4 * (g + 1)],
                                                  wqr[:, 4 * g:4 * (g + 1)])
                            for g in range(2):
                                nc.sync.dma_start(wkk_t[:, 8 * g:8 * (g + 1)],
                                                  wkk[:, 8 * g:8 * (g + 1)])
                            norm_chunk("q", QLR // P, 0, qan, QLR, 1)
                            norm_chunk("kv", KVLR // P, QLR, kvan, KVLR, 1)
                            kpe_chunk(1)
                            qn_sweep(1)

                        # w_q_b rope projection (m-outer, qc ping-pong)
                        with tc.tile_pool(name="psB1", bufs=1,
                                          space="PSUM") as psb1:
                            for m in range(HPC * DR // P):
                                ps2 = [psb1.tile([P, CW], F32, tag=f"psb{qc}",
                                                 name=f"psb{qc}", bufs=2)
                                       for qc in range(2)]
                                for k in range(QLR // P):
                                    for qc in range(2):
                                        nc.tensor.matmul(
                                            ps2[qc][:], wqr_t[:, m, k, :],
                                            qan[k][:, CW * qc:CW * (qc + 1)],
                                            start=(k == 0),
                                            stop=(k == QLR // P - 1))
                                for qc in range(2):
                                    nc.vector.tensor_mul(
                                        qr_t[m][:, CW * qc:CW * (qc + 1)],
                                        ps2[qc][:],
                                        rbs["q"][:, CW * qc:CW * (qc + 1)])
                        bw1.release()
                    # apq closed: qan freed

                    knv = tc.alloc_tile_pool(name="knv", bufs=1)
                    kn_t = [knv.tile([P, T], BF16, tag=f"kn{m}", name=f"kn{m}")
                            for m in range(HPC)]
                    v_sb = [knv.tile([P, HPC * DV], BF16, tag=f"v{tg}",
                                     name=f"v{tg}") for tg in range(T // P)]

                    # wv loads here: ~25us before the v matmuls need them
                    bwv = tc.alloc_tile_pool(name="bwv", bufs=1, side="right")
                    wv_t = bwv.tile([P, KVLR // P, HPC * DV], BF16, tag="wv",
                                    name="wv")
                    for g in range(2):
                        nc.sync.dma_start(wv_t[:, 2 * g:2 * (g + 1), :],
                                          wkv[:, 2 * g:2 * (g + 1), :])

                    with tc.tile_pool(name="psB2", bufs=1,
                                      space="PSUM") as psb2:
                        # scale kvan in place first: kn and v then both use
                        # scaled activations, and their drains are plain ACT
                        # copies (keeps the DVE free for the q-rope)
                        for k in range(KVLR // P):
                            nc.vector.tensor_mul(kvan[k][:], kvan[k][:],
                                                 rbs["kv"][:])

                        # kn projection (weights already resident)
                        for m in range(HPC):
                            ps2 = [psb2.tile([P, CW], F32, tag=f"psk{qc}",
                                             name=f"psk{qc}", bufs=2)
                                   for qc in range(2)]
                            for k in range(KVLR // P):
                                for qc in range(2):
                                    nc.tensor.matmul(
                                        ps2[qc][:], wkk_t[:, m, k, :],
                                        kvan[k][:, CW * qc:CW * (qc + 1)],
                                        start=(k == 0),
                                        stop=(k == KVLR // P - 1))
                            for qc in range(2):
                                nc.scalar.copy(
                                    kn_t[m][:, CW * qc:CW * (qc + 1)],
                                    ps2[qc][:])

                        # v (token-major): two token-tiles accumulate in
                        # alternating banks; ACT copy drain
                        for n4 in range(HPC * DV // CW):
                            for tg0 in range(0, T // P, 2):
                                ps = [psb2.tile([P, CW], F32, tag=f"psv{p}",
                                                name=f"psv{p}", bufs=2)
                                      for p in range(2)]
                                for k in range(KVLR // P):
                                    for p in range(2):
                                        nc.tensor.matmul(
                                            ps[p][:],
                                            kvan[k][:, P * (tg0 + p):
                                                     P * (tg0 + p + 1)],
                                            wv_t[:, k, CW * n4:CW * (n4 + 1)],
                                            start=(k == 0),
                                            stop=(k == KVLR // P - 1))
                                for p in range(2):
                                    nc.scalar.copy(
                                        v_sb[tg0 + p][:, CW * n4:CW * (n4 + 1)],
                                        ps[p][:])

                        # q-rope emitted last: its DVE work overlaps the v
                        # matmuls; attention consumes qr_t in m-order
                        for m in range(HPC * DR // P):
                            for cc in range(NCH):
                                rope_tile(qr_t[m], slice(CW * cc, CW * (cc + 1)))

                    bwv.release()
                    sn.release()
                    apk.release()

                    # o_proj weight pool opens below the attention pools on
                    # the right stack; the first loads fly during attention
                    sd = tc.alloc_tile_pool(name="stD", bufs=3, side="right")
                    WO_PF = 3
                    wo_q = []
                    for m in range(WO_PF):
                        wt = sd.tile([P, KT, P], BF16, tag="wo", name="wo",
                                     bufs=WO_PF)
                        nc.sync.dma_start(wt[:], wo[m])
                        wo_q.append(wt)

                    # -------- stage C: attention, heads in pairs --------
                    NT = T // P  # 8 k/q tiles
                    with (
                        tc.tile_pool(name="stC", bufs=2, side="right") as sc,
                        tc.tile_pool(name="ptP", bufs=2, side="right") as ptp,
                        tc.tile_pool(name="psS", bufs=1, space="PSUM") as pss,
                        tc.tile_pool(name="psO", bufs=1, space="PSUM") as pso,
                    ):
                        for hp in range(HPC // 2):
                            pair = (2 * hp, 2 * hp + 1)
                            qrs = {}
                            kps = {}
                            pts = {h: [] for h in pair}
                            for h in pair:
                                qm, qoff = divmod(DR * h, P)
                                qrs[h] = qr_t[qm][qoff:qoff + DR]
                                kps[h] = kpe2_t[qoff:qoff + DR]
                            for j in range(NT):
                                sp2 = {}
                                for h in pair:
                                    pts[h].append(
                                        ptp.tile([P, T - P * j], BF16,
                                                 tag=f"pt{j}", name=f"pt{j}"))
                                    sp2[h] = pss.tile([P, T], F32, tag="sps",
                                                      name="sps", bufs=3)
                                # all nope matmuls, then all rope matmuls
                                # (fewer PE tile-config switches); qc ranges
                                # split across the two banks of each tile
                                for h in pair:
                                    for qc in range(2):
                                        lo = max(CW * qc, P * j)
                                        hi = CW * (qc + 1)
                                        if lo >= hi:
                                            continue
                                        nc.tensor.matmul(
                                            sp2[h][:, lo:hi],
                                            kn_t[h][:, P * j:P * (j + 1)],
                                            qn_t[h][:, lo:hi],
                                            start=True, stop=False)
                                for h in pair:
                                    _, qoff = divmod(DR * h, P)
                                    for qc in range(2):
                                        lo = max(CW * qc, P * j)
                                        hi = CW * (qc + 1)
                                        if lo >= hi:
                                            continue
                                        nc.tensor.matmul(
                                            sp2[h][:, lo:hi],
                                            kps[h][:, P * j:P * (j + 1)],
                                            qrs[h][:, lo:hi],
                                            start=False, stop=True,
                                            tile_position=(qoff, 0))
                                    nc.scalar.activation(
                                        pts[h][-1][:], sp2[h][:, P * j:T],
                                        AF.Exp, bias=negm_t[:], scale=SM_SCALE)
                                    nc.vector.tensor_mul(pts[h][-1][:, :P],
                                                         pts[h][-1][:, :P],
                                                         triu_t[:])
                            rbis = {}
                            for h in pair:
                                # row sums over k (partition dim) via ones-
                                # matmul, j-outer with qc bank ping-pong
                                rps = [pso.tile([1, CW], F32, tag="ops",
                                                name=f"rps{qc}", bufs=2)
                                       for qc in range(2)]
                                for j in range(NT):
                                    for qc in range(2):
                                        lo = max(CW * qc, P * j)
                                        hi = CW * (qc + 1)
                                        if lo >= hi:
                                            continue
                                        jmax = 4 * (qc + 1)
                                        nc.tensor.matmul(
                                            rps[qc][:, lo - CW * qc:hi - CW * qc],
                                            ones_t[:],
                                            pts[h][j][:, lo - P * j:hi - P * j],
                                            start=(j == 0), stop=(j == jmax - 1))
                                rsb = sc.tile([1, T], F32, tag="rsb", name="rsb")
                                for qc in range(2):
                                    nc.vector.tensor_copy(
                                        rsb[:, CW * qc:CW * (qc + 1)],
                                        rps[qc][:])
                                rb = sc.tile([P, T], F32, tag="rbh", name="rbh")
                                nc.gpsimd.partition_broadcast(rb[:], rsb[:])
                                nc.vector.reciprocal_approx_fast(rb[:], rb[:])
                                rbis[h] = rb
                            for h in pair:
                                # attn @ v, j-outer qc ping-pong; normalized
                                # on the drain
                                ops2 = [pso.tile([P, CW], F32, tag="ops",
                                                 name=f"ops{qc}", bufs=2)
                                        for qc in range(2)]
                                for j in range(NT):
                                    for qc in range(2):
                                        lo = max(CW * qc, P * j)
                                        hi = CW * (qc + 1)
                                        if lo >= hi:
                                            continue
                                        jmax = 4 * (qc + 1)
                                        nc.tensor.matmul(
                                            ops2[qc][:, lo - CW * qc:],
                                            v_sb[j][:, DV * h:DV * (h + 1)],
                                            pts[h][j][:, lo - P * j:hi - P * j],
                                            start=(j == 0), stop=(j == jmax - 1))
                                for qc in range(2):
                                    nc.vector.tensor_mul(
                                        aot[:, h, CW * qc:CW * (qc + 1)],
                                        ops2[qc][:],
                                        rbis[h][:, CW * qc:CW * (qc + 1)])

                    knv.release()

                # ---------------- stage D: o_proj + RS --------
                # second wo pool in the space attention freed: deepens the
                # prefetch to ride out the ReduceScatter DMA windows
                sd2 = tc.alloc_tile_pool(name="stD2", bufs=1, side="right")
                WO_PF2 = 8
                for m in range(WO_PF, WO_PF2):
                    wt = sd2.tile([P, KT, P], BF16, tag="wo2", name="wo2",
                                  bufs=WO_PF2 - WO_PF)
                    nc.sync.dma_start(wt[:], wo[m])
                    wo_q.append(wt)
                with tc.tile_pool(name="psD", bufs=1, space="PSUM") as psd:
                    obase = 0
                    for r, mt in enumerate(RS_MT):
                        for mi in range(mt):
                            m = sum(RS_MT[:r]) + mi
                            wt = wo_q.pop(0)
                            if m + WO_PF2 < HID // P:
                                nwt = sd2.tile([P, KT, P], BF16, tag="wo2",
                                               name="wo2",
                                               bufs=WO_PF2 - WO_PF)
                                nc.sync.dma_start(nwt[:], wo[m + WO_PF2])
                                wo_q.append(nwt)
                            osb = sd.tile([P, T], BF16, tag="osb", name="osb",
                                          bufs=4)
                            ps2 = [psd.tile([P, CW], F32, tag=f"psd{qc}",
                                            name=f"psd{qc}", bufs=3)
                                   for qc in range(2)]
                            for k in range(KT):
                                for qc in range(2):
                                    nc.tensor.matmul(
                                        ps2[qc][:], wt[:, k, :],
                                        aot[:, k, CW * qc:CW * (qc + 1)],
                                        start=(k == 0), stop=(k == KT - 1))
                            for qc in range(2):
                                nc.scalar.copy(osb[:, CW * qc:CW * (qc + 1)],
                                               ps2[qc][:])
                            nc.sync.dma_start(o_dram[r][P * mi:P * (mi + 1), :],
                                              osb[:])
                        nc.gpsimd.collective_compute(
                            "ReduceScatter", mybir.AluOpType.add,
                            replica_groups=GROUPS,
                            ins=[o_dram[r][:]], outs=[rs_out[r][:]])
                        orows = mt * P // NC
                        nc.sync.dma_start(out_part[obase:obase + orows, :],
                                          rs_out[r][:])
                        obase += orows
                    sd2.release()
                    sd.release()

    nc.finalize()
    return nc


def _bf16(x):
    return np.ascontiguousarray(x.astype(ml_dtypes.bfloat16))


def _prep_inputs(positions, hidden_states, w_qkv_a, q_a_ln_w, w_q_b, kv_a_ln_w,
                 w_kv_b, w_o):
    positions = np.asarray(positions)
    hidden_states = np.asarray(hidden_states, dtype=np.float32)
    w_qkv_a = np.asarray(w_qkv_a, dtype=np.float32)
    q_a_ln_w = np.asarray(q_a_ln_w, dtype=np.float32)
    w_q_b = np.asarray(w_q_b, dtype=np.float32)
    kv_a_ln_w = np.asarray(kv_a_ln_w, dtype=np.float32)
    w_kv_b = np.asarray(w_kv_b, dtype=np.float32)
    w_o = np.asarray(w_o, dtype=np.float32)

    perm = np.concatenate([np.arange(0, DR, 2), np.arange(1, DR, 2)])

    # sign-folded rope tables, broadcast to 4x32-row groups [e,o,e,o]
    inv_freq = 1.0 / (THETA ** (np.arange(0, DR, 2, dtype=np.float32) / DR))
    freqs = positions.astype(np.float32)[:, None] * inv_freq  # [T, 32]
    cos = np.cos(freqs).T                                     # [32, T]
    sin = np.sin(freqs).T
    cos2 = _bf16(np.concatenate([cos, cos, cos, cos], axis=0))
    sin2 = _bf16(np.concatenate([-sin, sin, -sin, sin], axis=0))

    wa_full = w_qkv_a.copy()
    wa_full[:, QLR + KVLR:] = wa_full[:, QLR + KVLR:][:, perm]

    # hT p-major contiguous: hTt[p, k, t] = h[t, 128k+p]
    hTt = _bf16(hidden_states.T.reshape(NKH, P, T).transpose(1, 0, 2))
    wqb = (w_q_b * q_a_ln_w[:, None]).reshape(QLR, NH, DN + DR)
    wkvb = (w_kv_b * kv_a_ln_w[:, None]).reshape(KVLR, NH, DN + DV)

    triu_m = _bf16(np.triu(np.ones((P, P), dtype=np.float32)))
    ones_c = _bf16(np.ones((P, 1), dtype=np.float32))

    def tile_pkm(w):
        # [K, M] -> [P, M//P, K//P, P]: out[p, m, k, c] = w[P*k+p, P*m+c]
        K, M = w.shape
        return w.reshape(K // P, P, M // P, P).transpose(1, 2, 0, 3)

    def tile_k(w):
        # [K, M] -> [P, K//P, M]: out[p, k, c] = w[P*k+p, c]
        K, M = w.shape
        return w.reshape(K // P, P, M).transpose(1, 0, 2)

    def tile_km(w):
        # [K, M] -> [M//P, P, K//P, P]: out[m, p, k, c] = w[P*k+p, P*m+c]
        K, M = w.shape
        return w.reshape(K // P, P, M // P, P).transpose(2, 1, 0, 3)

    in_maps = []
    for c in range(NC):
        hs = slice(HPC * c, HPC * (c + 1))
        wo_sl = w_o[HPC * DV * c:HPC * DV * (c + 1), :]
        in_maps.append({
            "hT": hTt,
            "wa": _bf16(tile_k(wa_full[:, ACOL * c:ACOL * (c + 1)])),
            "cos2d": cos2,
            "sin2d": sin2,
            "wqn": _bf16(tile_pkm(wqb[:, hs, :DN].reshape(QLR, HPC * DN))),
            "wqr": _bf16(tile_pkm(wqb[:, hs, DN:][:, :, perm].reshape(QLR, HPC * DR))),
            "wkk": _bf16(tile_pkm(wkvb[:, hs, :DN].reshape(KVLR, HPC * DN))),
            "wkv": _bf16(tile_k(wkvb[:, hs, DN:].reshape(KVLR, HPC * DV))),
            "wo": _bf16(tile_km(wo_sl)),
            "triu": triu_m,
            "ones": ones_c,
        })
    return in_maps


def kernel(**inputs) -> np.ndarray:
    if "nc" not in _CACHE:
        _CACHE["nc"] = build()
    nc = _CACHE["nc"]
    in_maps = _prep_inputs(**inputs)
    res = run_bass_kernel_spmd(nc, in_maps, list(range(NC)))
    parts = np.stack([np.asarray(res.results[c]["out_part"]).astype(np.float32)
                      for c in range(NC)])
    # uneven RS chunks: chunk r spans rows [base_r, base_r + 128*mt[r]) of the
    # full output; core c holds its 1/NC slice of each chunk, concatenated.
    o = np.empty((HID, T), dtype=np.float32)
    base = 0
    obase = 0
    for mt in RS_MT:
        rows = mt * P
        sl = rows // NC
        for c in range(NC):
            o[base + sl * c:base + sl * (c + 1)] = parts[c][obase:obase + sl]
        base += rows
        obase += sl
    return np.ascontiguousarray(o.T)           # [T, HID]
